# revision 18
# baseline (speedup 1.0000x reference)
"""SkipGram negative-sampling loss kernel for 8 Trainium2 NeuronCores.

Strategy: data-parallel over walks (batch). The 1M x 128 embedding table is
replicated to every core's HBM; each core handles B/8 = 128 walks (one walk
per SBUF partition):
  - 6 large indirect-DMA gathers (walk split in two for an earlier compute
    start; one per neg plane) with f32->bf16 cast during DMA. Large gathers
    amortize the ~1us SWDGE per-instruction overhead.
  - dot products in 16 half-plane chunks (38 anchors each): bf16
    tensor_tensor multiply (2x DVE) + halving-add tree; the first tree level
    of the first POOL_N chunks runs on the otherwise-idle GpSimd engine.
  - softplus via the native ACT Softplus table (preloaded at t=0), one call
    per chunk with accum_out -> per-chunk partial sums; tiny final reduce.
  - each core returns [128, 1] partial sums; host sums and divides.
"""

import sys
import types

import numpy as np

try:  # missing in some containers; shim so trace=True degrades gracefully
    from antenv.axon_hooks import get_axon_ntff_profile_hook  # noqa: F401
except Exception:
    _m = types.ModuleType("antenv.axon_hooks")
    _m.get_axon_ntff_profile_hook = lambda: None
    sys.modules["antenv.axon_hooks"] = _m

import concourse.bass as bass
import concourse.bacc as bacc
import concourse.tile as tile
import concourse.mybir as mybir
from concourse.bass_utils import run_bass_kernel_spmd

F32 = mybir.dt.float32
BF16 = mybir.dt.bfloat16
I32 = mybir.dt.int32

N_CORES = 8
POOL_N = 8  # number of half-plane chunks whose tree level 1 runs on GpSimd
WALK_SPLIT = 42  # walk gather split column (first part covers half-0 pos work)


def build_kernel(n_walks, L, A, NEG, D, n_nodes, n_cores=N_CORES):
    """Build the SPMD Bass module (same NEFF on every core)."""
    W1 = L - A  # window_size - 1 = number of pos offsets (4)
    H = A // 2  # anchors per half-plane chunk (38)
    NCH = 2 * (W1 + NEG)  # 16 chunks
    nc = bacc.Bacc(
        "TRN2",
        target_bir_lowering=False,
        debug=False,
        num_devices=n_cores,
    )
    walk_idx = nc.dram_tensor("walk_idx", [n_walks, L], I32, kind="ExternalInput")
    neg_idx = nc.dram_tensor("neg_idx", [n_walks, NEG * A], I32, kind="ExternalInput")
    embed = nc.dram_tensor("embed", [n_nodes, D], F32, kind="ExternalInput")
    out = nc.dram_tensor("out", [n_walks, 1], F32, kind="ExternalOutput")

    with tile.TileContext(nc) as tc:
        with (
            tc.tile_pool(name="idx", bufs=1) as idxp,
            tc.tile_pool(name="ew", bufs=1) as ewp,
            tc.tile_pool(name="en", bufs=4) as enp,
            tc.tile_pool(name="prod", bufs=6) as prodp,
            tc.tile_pool(name="t1", bufs=3) as t1p,
            tc.tile_pool(name="t2", bufs=2) as t2p,
            tc.tile_pool(name="sp", bufs=1) as spp,
            tc.tile_pool(name="small", bufs=1) as smallp,
        ):
            # Abs table warmup: tiny activation at t=0 so the first ACT table
            # load happens during the gather phase, not on the critical tail
            warm = smallp.tile([n_walks, 4], F32)
            warm2 = smallp.tile([n_walks, 4], F32)
            nc.gpsimd.memset(warm[:], 0.0)
            nc.scalar.activation(
                warm2[:], warm[:], mybir.ActivationFunctionType.Abs
            )

            wi = idxp.tile([n_walks, L], I32)
            nc.sync.dma_start(out=wi[:], in_=walk_idx[:])
            ni = idxp.tile([n_walks, NEG * A], I32)
            nc.sync.dma_start(out=ni[:], in_=neg_idx[:])

            # walk gather, split so early pos chunks can start sooner
            ew16 = ewp.tile([n_walks, L * D], BF16)
            nc.gpsimd.indirect_dma_start(
                out=ew16[:, 0 : WALK_SPLIT * D],
                out_offset=None,
                in_=embed[:],
                in_offset=bass.IndirectOffsetOnAxis(ap=wi[:, 0:WALK_SPLIT], axis=0),
            )
            nc.gpsimd.indirect_dma_start(
                out=ew16[:, WALK_SPLIT * D :],
                out_offset=None,
                in_=embed[:],
                in_offset=bass.IndirectOffsetOnAxis(ap=wi[:, WALK_SPLIT:L], axis=0),
            )
            # one gather per neg plane
            en16 = []
            for j in range(NEG):
                t = enp.tile([n_walks, A * D], BF16)
                nc.gpsimd.indirect_dma_start(
                    out=t[:],
                    out_offset=None,
                    in_=embed[:],
                    in_offset=bass.IndirectOffsetOnAxis(
                        ap=ni[:, j * A : (j + 1) * A], axis=0
                    ),
                )
                en16.append(t)

            logits = smallp.tile([n_walks, NCH * H], F32)

            # chunk list: (anchor_col, other_ap, sign). Order: pos half-0
            # (needs only walk part 1), pos half-1, then neg planes by half.
            chunks = []
            for i in range(1, W1 + 1):
                chunks.append((0, ew16[:, i * D : (i + H) * D], -1.0))
            for i in range(1, W1 + 1):
                chunks.append(
                    (H, ew16[:, (i + H) * D : (i + H + H) * D], -1.0)
                )
            for j in range(NEG):
                for h in range(2):
                    chunks.append(
                        (h * H, en16[j][:, h * H * D : (h + 1) * H * D], 1.0)
                    )

            def mult(k):
                a0, other, _ = chunks[k]
                prod = prodp.tile([n_walks, H * D], BF16)
                nc.vector.tensor_mul(
                    prod[:], ew16[:, a0 * D : (a0 + H) * D], other
                )
                return prod

            def tail(k, prod):
                cur = prod[:].rearrange("p (a d) -> p a d", d=D)
                w = D
                eng = nc.gpsimd if k < POOL_N else nc.vector
                while w > 8:
                    h2 = w // 2
                    pool = t1p if w == D else t2p
                    nt = pool.tile([n_walks, H * h2], BF16, tag=f"t{h2}")
                    n3 = nt[:].rearrange("p (a d) -> p a d", d=h2)
                    eng.tensor_add(n3, cur[:, :, 0:h2], cur[:, :, h2:w])
                    eng = nc.vector  # only level 1 may run on GpSimd
                    cur = n3
                    w = h2
                nc.vector.tensor_reduce(
                    logits[:, k * H : (k + 1) * H],
                    cur,
                    axis=mybir.AxisListType.X,
                    op=mybir.AluOpType.add,
                )

            # interleave: tail(k) is emitted two multiplies later so the
            # GpSimd level-1 of chunk k overlaps DVE multiplies k+1, k+2
            prods = {}
            LAG = 2
            for k in range(NCH):
                prods[k] = mult(k)
                if k - LAG >= 0:
                    tail(k - LAG, prods.pop(k - LAG))
            for k in range(NCH - LAG, NCH):
                tail(k, prods.pop(k))

            # batched stable softplus over all NCH*H logits:
            #   softplus(s*x) = max(s*x, 0) + ln(1 + exp(-|x|))
            # (s = -1 for pos chunks [0, NCH/2), +1 for neg chunks)
            NL = NCH * H
            PL = (NCH // 2) * H  # pos block size
            rl = spp.tile([n_walks, NL], F32)
            nc.vector.tensor_scalar(
                rl[:, 0:PL], logits[:, 0:PL], -1.0, 0.0,
                mybir.AluOpType.mult, mybir.AluOpType.max,
            )
            nc.vector.tensor_scalar(
                rl[:, PL:NL], logits[:, PL:NL], 1.0, 0.0,
                mybir.AluOpType.mult, mybir.AluOpType.max,
            )
            ab = spp.tile([n_walks, NL], F32)
            nc.scalar.activation(ab[:], logits[:], mybir.ActivationFunctionType.Abs)
            e = spp.tile([n_walks, NL], F32)
            nc.scalar.activation(
                e[:], ab[:], mybir.ActivationFunctionType.Exp, scale=-1.0
            )
            ln1 = spp.tile([n_walks, NL], F32)
            nc.scalar.activation(
                ln1[:], e[:], mybir.ActivationFunctionType.Ln, bias=1.0
            )
            sp = spp.tile([n_walks, NL], F32)
            nc.vector.tensor_add(sp[:], ln1[:], rl[:])
            osum = smallp.tile([n_walks, 1], F32)
            nc.vector.tensor_reduce(
                osum[:],
                sp[:],
                axis=mybir.AxisListType.X,
                op=mybir.AluOpType.add,
            )
            nc.sync.dma_start(out=out[:], in_=osum[:])

    nc.compile()
    return nc


_NC_CACHE = {}


def _get_nc(key):
    if key not in _NC_CACHE:
        _NC_CACHE[key] = build_kernel(*key)
    return _NC_CACHE[key]


def make_in_maps(walk, neg, embed, n_cores=N_CORES):
    B, L = walk.shape
    A, NEG = neg.shape[1], neg.shape[2]
    nw = B // n_cores
    embed_f = np.ascontiguousarray(embed.astype(np.float32, copy=False))
    in_maps = []
    for c in range(n_cores):
        sl = slice(c * nw, (c + 1) * nw)
        wslice = np.ascontiguousarray(walk[sl].astype(np.int32, copy=False))
        # neg [nw, A, NEG] -> plane-major [nw, NEG*A]
        nslice = np.ascontiguousarray(
            neg[sl].astype(np.int32, copy=False).transpose(0, 2, 1).reshape(nw, NEG * A)
        )
        in_maps.append({"walk_idx": wslice, "neg_idx": nslice, "embed": embed_f})
    return in_maps


def kernel(walk, neg, embed, _trace=False):
    walk = np.asarray(walk)
    neg = np.asarray(neg)
    embed = np.asarray(embed)
    B, L = walk.shape
    A, NEG = neg.shape[1], neg.shape[2]
    n_nodes, D = embed.shape

    nc = _get_nc((B // N_CORES, L, A, NEG, D, n_nodes, N_CORES))
    in_maps = make_in_maps(walk, neg, embed)
    res = run_bass_kernel_spmd(
        nc, in_maps, core_ids=list(range(N_CORES)), trace=_trace
    )
    total = 2 * B * A * NEG
    s = sum(r["out"].astype(np.float64).sum() for r in res.results)
    loss = np.float32(s / total)
    if _trace:
        return loss, res
    return loss


# revision 24
# speedup vs baseline: 1.1558x; 1.1558x over previous
"""SkipGram negative-sampling loss kernel for 8 Trainium2 NeuronCores.

Strategy: data-parallel over walks (batch). The 1M x 128 embedding table is
replicated to every core's HBM; each core handles B/8 = 128 walks (one walk
per SBUF partition):
  - 6 large indirect-DMA gathers (walk split in two for an earlier compute
    start; one per neg plane) with f32->bf16 cast during DMA. Large gathers
    amortize the ~1us SWDGE per-instruction overhead.
  - dot products in 16 half-plane chunks (38 anchors each): bf16
    tensor_tensor multiply (2x DVE) + halving-add tree; the first tree level
    of the first POOL_N chunks runs on the otherwise-idle GpSimd engine.
  - softplus via the native ACT Softplus table (preloaded at t=0), one call
    per chunk with accum_out -> per-chunk partial sums; tiny final reduce.
  - each core returns [128, 1] partial sums; host sums and divides.
"""

import sys
import types

import numpy as np

try:  # missing in some containers; shim so trace=True degrades gracefully
    from antenv.axon_hooks import get_axon_ntff_profile_hook  # noqa: F401
except Exception:
    _m = types.ModuleType("antenv.axon_hooks")
    _m.get_axon_ntff_profile_hook = lambda: None
    sys.modules["antenv.axon_hooks"] = _m

import concourse.bass as bass
import concourse.bacc as bacc
import concourse.tile as tile
import concourse.mybir as mybir
from concourse.bass_utils import run_bass_kernel_spmd

F32 = mybir.dt.float32
BF16 = mybir.dt.bfloat16
I32 = mybir.dt.int32

N_CORES = 8
POOL_N = 0  # number of chunks whose tree level 1 runs on GpSimd (HW: too slow)
WALK_SPLIT = 0  # unused with full-plane chunks


def build_kernel(n_walks, L, A, NEG, D, n_nodes, n_cores=N_CORES):
    """Build the SPMD Bass module (same NEFF on every core)."""
    W1 = L - A  # window_size - 1 = number of pos offsets (4)
    H = A  # anchors per chunk (full planes; halving costs more in overhead)
    NCH = W1 + NEG  # 8 chunks
    nc = bacc.Bacc(
        "TRN2",
        target_bir_lowering=False,
        debug=False,
        num_devices=n_cores,
    )
    walk_idx = nc.dram_tensor("walk_idx", [n_walks, L], I32, kind="ExternalInput")
    neg_idx = nc.dram_tensor("neg_idx", [n_walks, NEG * A], I32, kind="ExternalInput")
    embed = nc.dram_tensor("embed", [n_nodes, D], F32, kind="ExternalInput")
    out = nc.dram_tensor("out", [n_walks, 1], F32, kind="ExternalOutput")

    with tile.TileContext(nc) as tc:
        with (
            tc.tile_pool(name="idx", bufs=1) as idxp,
            tc.tile_pool(name="ew", bufs=1) as ewp,
            tc.tile_pool(name="en", bufs=4) as enp,
            tc.tile_pool(name="prod", bufs=3) as prodp,
            tc.tile_pool(name="t1", bufs=2) as t1p,
            tc.tile_pool(name="t2", bufs=2) as t2p,
            tc.tile_pool(name="sp", bufs=1) as spp,
            tc.tile_pool(name="small", bufs=1) as smallp,
        ):
            # Abs table warmup: tiny activation at t=0 so the first ACT table
            # load happens during the gather phase, not on the critical tail
            warm = smallp.tile([n_walks, 4], F32)
            warm2 = smallp.tile([n_walks, 4], F32)
            nc.gpsimd.memset(warm[:], 0.0)
            nc.scalar.activation(
                warm2[:], warm[:], mybir.ActivationFunctionType.Abs
            )

            wi = idxp.tile([n_walks, L], I32)
            nc.sync.dma_start(out=wi[:], in_=walk_idx[:])
            ni = idxp.tile([n_walks, NEG * A], I32)
            nc.sync.dma_start(out=ni[:], in_=neg_idx[:])

            # one big gather for the whole walk (L rows per partition)
            ew16 = ewp.tile([n_walks, L * D], BF16)
            nc.gpsimd.indirect_dma_start(
                out=ew16[:],
                out_offset=None,
                in_=embed[:],
                in_offset=bass.IndirectOffsetOnAxis(ap=wi[:, 0:L], axis=0),
            )
            # one gather per neg plane
            en16 = []
            for j in range(NEG):
                t = enp.tile([n_walks, A * D], BF16)
                nc.gpsimd.indirect_dma_start(
                    out=t[:],
                    out_offset=None,
                    in_=embed[:],
                    in_offset=bass.IndirectOffsetOnAxis(
                        ap=ni[:, j * A : (j + 1) * A], axis=0
                    ),
                )
                en16.append(t)

            logits = smallp.tile([n_walks, NCH * H], F32)

            # chunk list: (anchor_col, other_ap, sign): pos planes then neg.
            chunks = []
            for i in range(1, W1 + 1):
                chunks.append((0, ew16[:, i * D : (i + A) * D], -1.0))
            for j in range(NEG):
                chunks.append((0, en16[j][:], 1.0))

            def mult(k):
                a0, other, _ = chunks[k]
                prod = prodp.tile([n_walks, H * D], BF16)
                nc.vector.tensor_mul(
                    prod[:], ew16[:, a0 * D : (a0 + H) * D], other
                )
                return prod

            def tail(k, prod):
                cur = prod[:].rearrange("p (a d) -> p a d", d=D)
                w = D
                eng = nc.gpsimd if k < POOL_N else nc.vector
                while w > 8:
                    h2 = w // 2
                    pool = t1p if w == D else t2p
                    nt = pool.tile([n_walks, H * h2], BF16, tag=f"t{h2}")
                    n3 = nt[:].rearrange("p (a d) -> p a d", d=h2)
                    eng.tensor_add(n3, cur[:, :, 0:h2], cur[:, :, h2:w])
                    eng = nc.vector  # only level 1 may run on GpSimd
                    cur = n3
                    w = h2
                nc.vector.tensor_reduce(
                    logits[:, k * H : (k + 1) * H],
                    cur,
                    axis=mybir.AxisListType.X,
                    op=mybir.AluOpType.add,
                )

            # interleave: tail(k) is emitted two multiplies later so the
            # GpSimd level-1 of chunk k overlaps DVE multiplies k+1, k+2
            prods = {}
            LAG = 2
            for k in range(NCH):
                prods[k] = mult(k)
                if k - LAG >= 0:
                    tail(k - LAG, prods.pop(k - LAG))
            for k in range(NCH - LAG, NCH):
                tail(k, prods.pop(k))

            # batched stable softplus over all NCH*H logits:
            #   softplus(s*x) = max(s*x, 0) + ln(1 + exp(-|x|))
            # (s = -1 for pos chunks [0, NCH/2), +1 for neg chunks)
            NL = NCH * H
            PL = (NCH // 2) * H  # pos block size
            rl = spp.tile([n_walks, NL], F32)
            nc.vector.tensor_scalar(
                rl[:, 0:PL], logits[:, 0:PL], -1.0, 0.0,
                mybir.AluOpType.mult, mybir.AluOpType.max,
            )
            nc.vector.tensor_scalar(
                rl[:, PL:NL], logits[:, PL:NL], 1.0, 0.0,
                mybir.AluOpType.mult, mybir.AluOpType.max,
            )
            ab = spp.tile([n_walks, NL], F32)
            nc.scalar.activation(ab[:], logits[:], mybir.ActivationFunctionType.Abs)
            e = spp.tile([n_walks, NL], F32)
            nc.scalar.activation(
                e[:], ab[:], mybir.ActivationFunctionType.Exp, scale=-1.0
            )
            ln1 = spp.tile([n_walks, NL], F32)
            nc.scalar.activation(
                ln1[:], e[:], mybir.ActivationFunctionType.Ln, bias=1.0
            )
            sp = spp.tile([n_walks, NL], F32)
            nc.vector.tensor_add(sp[:], ln1[:], rl[:])
            osum = smallp.tile([n_walks, 1], F32)
            nc.vector.tensor_reduce(
                osum[:],
                sp[:],
                axis=mybir.AxisListType.X,
                op=mybir.AluOpType.add,
            )
            nc.sync.dma_start(out=out[:], in_=osum[:])

    nc.compile()
    return nc


_NC_CACHE = {}


def _get_nc(key):
    if key not in _NC_CACHE:
        _NC_CACHE[key] = build_kernel(*key)
    return _NC_CACHE[key]


def make_in_maps(walk, neg, embed, n_cores=N_CORES):
    B, L = walk.shape
    A, NEG = neg.shape[1], neg.shape[2]
    nw = B // n_cores
    embed_f = np.ascontiguousarray(embed.astype(np.float32, copy=False))
    in_maps = []
    for c in range(n_cores):
        sl = slice(c * nw, (c + 1) * nw)
        wslice = np.ascontiguousarray(walk[sl].astype(np.int32, copy=False))
        # neg [nw, A, NEG] -> plane-major [nw, NEG*A]
        nslice = np.ascontiguousarray(
            neg[sl].astype(np.int32, copy=False).transpose(0, 2, 1).reshape(nw, NEG * A)
        )
        in_maps.append({"walk_idx": wslice, "neg_idx": nslice, "embed": embed_f})
    return in_maps


def kernel(walk, neg, embed, _trace=False):
    walk = np.asarray(walk)
    neg = np.asarray(neg)
    embed = np.asarray(embed)
    B, L = walk.shape
    A, NEG = neg.shape[1], neg.shape[2]
    n_nodes, D = embed.shape

    nc = _get_nc((B // N_CORES, L, A, NEG, D, n_nodes, N_CORES))
    in_maps = make_in_maps(walk, neg, embed)
    res = run_bass_kernel_spmd(
        nc, in_maps, core_ids=list(range(N_CORES)), trace=_trace
    )
    total = 2 * B * A * NEG
    s = sum(r["out"].astype(np.float64).sum() for r in res.results)
    loss = np.float32(s / total)
    if _trace:
        return loss, res
    return loss


# revision 26
# speedup vs baseline: 1.2144x; 1.0506x over previous
"""SkipGram negative-sampling loss kernel for 8 Trainium2 NeuronCores.

Strategy: data-parallel over walks (batch). The 1M x 128 embedding table is
replicated to every core's HBM; each core handles B/8 = 128 walks (one walk
per SBUF partition):
  - 6 large indirect-DMA gathers (walk split in two for an earlier compute
    start; one per neg plane) with f32->bf16 cast during DMA. Large gathers
    amortize the ~1us SWDGE per-instruction overhead.
  - dot products in 16 half-plane chunks (38 anchors each): bf16
    tensor_tensor multiply (2x DVE) + halving-add tree; the first tree level
    of the first POOL_N chunks runs on the otherwise-idle GpSimd engine.
  - softplus via the native ACT Softplus table (preloaded at t=0), one call
    per chunk with accum_out -> per-chunk partial sums; tiny final reduce.
  - each core returns [128, 1] partial sums; host sums and divides.
"""

import sys
import types

import numpy as np

try:  # missing in some containers; shim so trace=True degrades gracefully
    from antenv.axon_hooks import get_axon_ntff_profile_hook  # noqa: F401
except Exception:
    _m = types.ModuleType("antenv.axon_hooks")
    _m.get_axon_ntff_profile_hook = lambda: None
    sys.modules["antenv.axon_hooks"] = _m

import concourse.bass as bass
import concourse.bacc as bacc
import concourse.tile as tile
import concourse.mybir as mybir
from concourse.bass_utils import run_bass_kernel_spmd

F32 = mybir.dt.float32
BF16 = mybir.dt.bfloat16
I32 = mybir.dt.int32

N_CORES = 8
POOL_N = 0  # number of chunks whose tree level 1 runs on GpSimd (HW: too slow)
WALK_SPLIT = 0  # unused with full-plane chunks


def build_kernel(n_walks, L, A, NEG, D, n_nodes, n_cores=N_CORES):
    """Build the SPMD Bass module (same NEFF on every core)."""
    W1 = L - A  # window_size - 1 = number of pos offsets (4)
    H = A  # anchors per chunk (full planes; halving costs more in overhead)
    NCH = W1 + NEG  # 8 chunks
    nc = bacc.Bacc(
        "TRN2",
        target_bir_lowering=False,
        debug=False,
        num_devices=n_cores,
    )
    walk_idx = nc.dram_tensor("walk_idx", [n_walks, L], I32, kind="ExternalInput")
    neg_idx = nc.dram_tensor("neg_idx", [n_walks, NEG * A], I32, kind="ExternalInput")
    embed = nc.dram_tensor("embed", [n_nodes, D], F32, kind="ExternalInput")
    out = nc.dram_tensor("out", [n_walks, 1], F32, kind="ExternalOutput")

    with tile.TileContext(nc) as tc:
        with (
            tc.tile_pool(name="idx", bufs=1) as idxp,
            tc.tile_pool(name="ew", bufs=1) as ewp,
            tc.tile_pool(name="en", bufs=4) as enp,
            tc.tile_pool(name="prod", bufs=3) as prodp,
            tc.tile_pool(name="t1", bufs=2) as t1p,
            tc.tile_pool(name="t2", bufs=2) as t2p,
            tc.tile_pool(name="sp", bufs=1) as spp,
            tc.tile_pool(name="small", bufs=1) as smallp,
        ):
            # Abs table warmup: tiny activation at t=0 so the first ACT table
            # load happens during the gather phase, not on the critical tail
            warm = smallp.tile([n_walks, 4], F32)
            warm2 = smallp.tile([n_walks, 4], F32)
            nc.gpsimd.memset(warm[:], 0.0)
            nc.scalar.activation(
                warm2[:], warm[:], mybir.ActivationFunctionType.Abs
            )

            wi = idxp.tile([n_walks, L], I32)
            nc.sync.dma_start(out=wi[:], in_=walk_idx[:])
            ni = idxp.tile([n_walks, NEG * A], I32)
            nc.sync.dma_start(out=ni[:], in_=neg_idx[:])

            # walk gather, split so the first pos half-plane can start early
            WS = 44
            ew16 = ewp.tile([n_walks, L * D], BF16)
            nc.gpsimd.indirect_dma_start(
                out=ew16[:, 0 : WS * D],
                out_offset=None,
                in_=embed[:],
                in_offset=bass.IndirectOffsetOnAxis(ap=wi[:, 0:WS], axis=0),
            )
            nc.gpsimd.indirect_dma_start(
                out=ew16[:, WS * D :],
                out_offset=None,
                in_=embed[:],
                in_offset=bass.IndirectOffsetOnAxis(ap=wi[:, WS:L], axis=0),
            )
            # one gather per neg plane
            en16 = []
            for j in range(NEG):
                t = enp.tile([n_walks, A * D], BF16)
                nc.gpsimd.indirect_dma_start(
                    out=t[:],
                    out_offset=None,
                    in_=embed[:],
                    in_offset=bass.IndirectOffsetOnAxis(
                        ap=ni[:, j * A : (j + 1) * A], axis=0
                    ),
                )
                en16.append(t)

            # chunk list: (anc_ap, other_ap, n_anchors). The first pos plane
            # is split at anchor 38 so chunk 0 only needs walk cols < WS.
            HA = 38
            chunks = [
                (ew16[:, 0 : HA * D], ew16[:, 1 * D : (1 + HA) * D], HA),
                (ew16[:, HA * D : A * D], ew16[:, (1 + HA) * D : (1 + A) * D], HA),
            ]
            for i in range(2, W1 + 1):
                chunks.append((ew16[:, 0 : A * D], ew16[:, i * D : (i + A) * D], A))
            for j in range(NEG):
                chunks.append((ew16[:, 0 : A * D], en16[j][:], A))
            NL = sum(c[2] for c in chunks)  # total logit columns
            PL = 2 * HA + (W1 - 1) * A  # pos block size
            offs = [0]
            for c in chunks:
                offs.append(offs[-1] + c[2])

            logits = smallp.tile([n_walks, NL], F32)

            def mult(k):
                anc, other, na = chunks[k]
                prod = prodp.tile([n_walks, A * D], BF16)
                nc.vector.tensor_mul(prod[:, 0 : na * D], anc, other)
                return prod

            def tail(k, prod):
                na = chunks[k][2]
                cur = prod[:, 0 : na * D].rearrange("p (a d) -> p a d", d=D)
                w = D
                while w > 4:
                    h2 = w // 2
                    pool = t1p if w == D else t2p
                    nt = pool.tile([n_walks, A * h2], BF16, tag=f"t{h2}")
                    n3 = nt[:, 0 : na * h2].rearrange("p (a d) -> p a d", d=h2)
                    nc.vector.tensor_add(n3, cur[:, :, 0:h2], cur[:, :, h2:w])
                    cur = n3
                    w = h2
                nc.vector.tensor_reduce(
                    logits[:, offs[k] : offs[k + 1]],
                    cur,
                    axis=mybir.AxisListType.X,
                    op=mybir.AluOpType.add,
                )

            # tail(k) is emitted two multiplies later to keep DVE fed while
            # gather-gated multiplies wait
            prods = {}
            NCHUNK = len(chunks)
            LAG = 2
            for k in range(NCHUNK):
                prods[k] = mult(k)
                if k - LAG >= 0:
                    tail(k - LAG, prods.pop(k - LAG))
            for k in range(NCHUNK - LAG, NCHUNK):
                tail(k, prods.pop(k))

            # batched stable softplus over all logits, fully on ACT:
            #   softplus(s*x) = relu(s*x) + ln(1 + exp(-|x|))
            # (s = -1 for pos chunks, +1 for neg). The three partial sums
            # land in accum columns; a tiny DVE reduce finishes.
            acc3 = smallp.tile([n_walks, 3], F32)
            scr = spp.tile([n_walks, NL], F32)
            nc.scalar.activation(
                scr[:, 0:PL], logits[:, 0:PL],
                mybir.ActivationFunctionType.Relu,
                scale=-1.0, accum_out=acc3[:, 0:1],
            )
            nc.scalar.activation(
                scr[:, PL:NL], logits[:, PL:NL],
                mybir.ActivationFunctionType.Relu,
                scale=1.0, accum_out=acc3[:, 1:2],
            )
            ab = spp.tile([n_walks, NL], F32)
            nc.scalar.activation(ab[:], logits[:], mybir.ActivationFunctionType.Abs)
            e = spp.tile([n_walks, NL], F32)
            nc.scalar.activation(
                e[:], ab[:], mybir.ActivationFunctionType.Exp, scale=-1.0
            )
            ln1 = spp.tile([n_walks, NL], F32)
            nc.scalar.activation(
                ln1[:], e[:], mybir.ActivationFunctionType.Ln, bias=1.0,
                accum_out=acc3[:, 2:3],
            )
            osum = smallp.tile([n_walks, 1], F32)
            nc.vector.tensor_reduce(
                osum[:],
                acc3[:],
                axis=mybir.AxisListType.X,
                op=mybir.AluOpType.add,
            )
            nc.sync.dma_start(out=out[:], in_=osum[:])

    nc.compile()
    return nc


_NC_CACHE = {}


def _get_nc(key):
    if key not in _NC_CACHE:
        _NC_CACHE[key] = build_kernel(*key)
    return _NC_CACHE[key]


def make_in_maps(walk, neg, embed, n_cores=N_CORES):
    B, L = walk.shape
    A, NEG = neg.shape[1], neg.shape[2]
    nw = B // n_cores
    embed_f = np.ascontiguousarray(embed.astype(np.float32, copy=False))
    in_maps = []
    for c in range(n_cores):
        sl = slice(c * nw, (c + 1) * nw)
        wslice = np.ascontiguousarray(walk[sl].astype(np.int32, copy=False))
        # neg [nw, A, NEG] -> plane-major [nw, NEG*A]
        nslice = np.ascontiguousarray(
            neg[sl].astype(np.int32, copy=False).transpose(0, 2, 1).reshape(nw, NEG * A)
        )
        in_maps.append({"walk_idx": wslice, "neg_idx": nslice, "embed": embed_f})
    return in_maps


def kernel(walk, neg, embed, _trace=False):
    walk = np.asarray(walk)
    neg = np.asarray(neg)
    embed = np.asarray(embed)
    B, L = walk.shape
    A, NEG = neg.shape[1], neg.shape[2]
    n_nodes, D = embed.shape

    nc = _get_nc((B // N_CORES, L, A, NEG, D, n_nodes, N_CORES))
    in_maps = make_in_maps(walk, neg, embed)
    res = run_bass_kernel_spmd(
        nc, in_maps, core_ids=list(range(N_CORES)), trace=_trace
    )
    total = 2 * B * A * NEG
    s = sum(r["out"].astype(np.float64).sum() for r in res.results)
    loss = np.float32(s / total)
    if _trace:
        return loss, res
    return loss
